# revision 1
# baseline (speedup 1.0000x reference)
"""DeepSeek-MoE layer Trainium2 Bass kernel.

Strategy: data-parallel over tokens. N = B*T = 4096 tokens are split into 8
chunks of 512 (each chunk belongs to a single batch row, so the t_emb
broadcast is per-chunk constant). Each of the 8 NeuronCores runs the
identical program on its chunk, with router/shared/expert weights replicated:

  - router: logits^T [E, n] via PE (x part N=512, t part N=1 + per-partition
    add), transpose to [n, E], sigmoid, top-2 via max_with_indices, gates
    normalized over selected raw affinities, gate columns pre-scaled by
    1/(N_SHARED+K).
  - shared expert: SwiGLU; its second matmul initializes the output
    accumulator in SBUF.
  - routed experts: evaluated densely (every expert over every local token);
    each expert's second matmul lands in PSUM and is fused into the SBUF
    accumulator as acc += gate_col[e] * psum (scalar_tensor_tensor), which
    zeroes non-selected tokens.

All activations live transposed ([feature, token]) so both matmul layers
consume them directly as PE operands with no on-chip transposes; the x^T
layout is prepared host-side. Output lands in natural [token, d] layout.
Matmul dtype switchable via MOE_MM_MODE: f32 (exact, 1/4-rate PE) or
f32r (full-rate PE, truncated mantissa).
"""

import os
import numpy as np
from contextlib import ExitStack

import concourse.bacc as bacc
import concourse.tile as tile
from concourse import mybir
from concourse.alu_op_type import AluOpType
from concourse.masks import make_identity
from concourse.bass_utils import run_bass_kernel_spmd

f32 = mybir.dt.float32

# problem shapes (hardcoded per contract)
D, HS, HE, E, TOPK = 1024, 2048, 1024, 8, 2
B, T = 4, 1024
NCORES = 8
NTOK = (B * T) // NCORES  # 512 tokens per core
P = 128
NT = NTOK // P  # 4 token tiles
KD = D // P     # 8 contraction chunks over D
KH = HE // P    # 8 contraction chunks over HE
KS = HS // P    # 16 contraction chunks over HS
OUT_SCALE = 1.0 / 3.0  # 1/(N_SHARED + TOPK)

MM_MODE = os.environ.get("MOE_MM_MODE", "f32")
MMDT = {"f32": mybir.dt.float32, "f32r": mybir.dt.float32r,
        "bf16": mybir.dt.bfloat16}[MM_MODE]
# CoreSim doesn't implement Silu/Gelu; compat mode composes them from
# Sigmoid (silu exactly; gelu via the 1.702-sigmoid approx — wiring check
# only, hardware always runs the real thing).
ACT_COMPAT = os.environ.get("MOE_ACT_COMPAT", "0") == "1"
PARTS = set(os.environ.get("MOE_PARTS", "router,shared,exp").split(","))
DMA_ENG = os.environ.get("MOE_DMA_ENG", "sync")
AF = mybir.ActivationFunctionType




def emit_body(nc, pools, dram, identity_tile):
    xp, tp, wk, g1p, hp, accp, rp, tmpp, xrp, ps_h, ps_o, ps_r = pools
    dma = getattr(nc, DMA_ENG)

    def act_silu(out_ap, ps_ap):
        if ACT_COMPAT:
            tmp = tmpp.tile([P, NTOK], f32, tag="tmp")
            nc.scalar.activation(tmp[:], ps_ap, AF.Sigmoid)
            nc.vector.tensor_tensor(out_ap, ps_ap, tmp[:], AluOpType.mult)
        else:
            nc.scalar.activation(out_ap, ps_ap, AF.Silu)

    def act_gelu(out_ap, ps_ap):
        if ACT_COMPAT:
            tmp = tmpp.tile([P, NTOK], f32, tag="tmp")
            nc.scalar.activation(tmp[:], ps_ap, AF.Sigmoid, scale=1.702)
            nc.vector.tensor_tensor(out_ap, ps_ap, tmp[:], AluOpType.mult)
        else:
            nc.scalar.activation(out_ap, ps_ap, AF.Gelu)

    xT = xp.tile([P, KD, NTOK], MMDT, tag="xT")
    dma.dma_start(out=xT[:], in_=dram["xT"].rearrange("(k p) n -> p k n", p=P))
    t_sb = tp.tile([P, KD], f32, tag="t_sb")
    dma.dma_start(out=t_sb[:], in_=dram["t_row"].rearrange("(k p) -> p k", p=P))

    # ---- router ----
    comb = rp.tile([P, NT * E], f32, tag="comb")
    if "router" not in PARTS:
        nc.vector.memset(comb[:], 0.125)
    if "router" in PARTS:
      rw_sb = rp.tile([P, 2 * KD, E], f32, tag="rw")
      dma.dma_start(
          out=rw_sb[:], in_=dram["router_W"].rearrange("(k p) e -> p k e", p=P)
      )
      bias_sb = rp.tile([P, NT * E], f32, tag="bias")
      dma.dma_start(out=bias_sb[:], in_=dram["router_bias_b"][:])

      psR = ps_r.tile([E, NTOK], f32, tag="psR")
      for k in range(KD):
          xr = xrp.tile([P, NTOK], f32, tag="xr")
          dma.dma_start(out=xr[:], in_=dram["x_rt"][k * P:(k + 1) * P, :])
          nc.tensor.matmul(
              psR[:], rw_sb[:, k, :], xr[:],
              start=(k == 0), stop=(k == KD - 1),
          )
      psRt = ps_r.tile([E, 1], f32, tag="psRt")
      for k in range(KD):
          nc.tensor.matmul(
              psRt[:], rw_sb[:, KD + k, :], t_sb[:, k:k + 1],
              start=(k == 0), stop=(k == KD - 1),
          )
      t_logit = rp.tile([E, 1], f32, tag="t_logit")
      nc.vector.tensor_copy(t_logit[:], psRt[:])
      logits_sb = rp.tile([E, NTOK], f32, tag="logits")
      nc.vector.tensor_scalar(
          out=logits_sb[:], in0=psR[:], scalar1=t_logit[:], scalar2=None,
          op0=AluOpType.add,
      )

      psT = ps_r.tile([P, NT * E], f32, tag="psT")
      for t in range(NT):
          nc.tensor.transpose(
              psT[:, t * E:(t + 1) * E],
              logits_sb[:, t * P:(t + 1) * P],
              identity_tile[:E, :E],
          )
      s_sb = rp.tile([P, NT * E], f32, tag="s")
      nc.scalar.activation(s_sb[:], psT[:], AF.Sigmoid)
      sel = rp.tile([P, NT * E], f32, tag="sel")
      nc.vector.tensor_tensor(sel[:], s_sb[:], bias_sb[:], AluOpType.add)

      mx = rp.tile([P, NT * E], f32, tag="mx")
      midx = rp.tile([P, NT * E], mybir.dt.uint32, tag="midx")
      mask = rp.tile([P, NT * E], f32, tag="mask")
      sgated = rp.tile([P, NT * E], f32, tag="sgated")
      denom = rp.tile([P, NT], f32, tag="denom")
      rec = rp.tile([P, NT], f32, tag="rec")
      for t in range(NT):
          sl = slice(t * E, (t + 1) * E)
          nc.vector.max_with_indices(mx[:, sl], midx[:, sl], sel[:, sl])
          nc.vector.tensor_scalar(
              out=mask[:, sl], in0=sel[:, sl],
              scalar1=mx[:, t * E + 1:t * E + 2], scalar2=None, op0=AluOpType.is_ge,
          )
          nc.vector.scalar_tensor_tensor(
              out=sgated[:, sl], in0=mask[:, sl], scalar=1.0, in1=s_sb[:, sl],
              op0=AluOpType.mult, op1=AluOpType.mult,
              accum_out=denom[:, t:t + 1],
          )
      nc.vector.tensor_scalar(
          out=denom[:], in0=denom[:], scalar1=1e-9, scalar2=None,
          op0=AluOpType.add,
      )
      nc.vector.reciprocal(out=rec[:], in_=denom[:])
      for t in range(NT):
          sl = slice(t * E, (t + 1) * E)
          nc.vector.tensor_scalar(
              out=comb[:, sl], in0=sgated[:, sl], scalar1=rec[:, t:t + 1],
              scalar2=OUT_SCALE, op0=AluOpType.mult, op1=AluOpType.mult,
          )

    routed = accp.tile([P, NT, D], f32, tag="routed")

    # ---- shared expert: g1s = silu(x @ w1); g1s *= (x @ w3); then @ w2 ----
    if "shared" not in PARTS:
        nc.vector.memset(routed[:], 0.0)
    if "shared" in PARTS:
      g1s = g1p.tile([P, KS, NTOK], MMDT, tag="g1s")
      for wname, is_first in (("w1", True), ("w3", False)):
          for half in range(2):
              wt = []
              for k in range(KD):
                  w = wk.tile([P, HS // 2], MMDT, tag="wk")
                  dma.dma_start(
                      out=w[:],
                      in_=dram[wname][k * P:(k + 1) * P,
                                      half * (HS // 2):(half + 1) * (HS // 2)],
                  )
                  wt.append(w)
              for j in range(KS // 2):
                  jj = half * (KS // 2) + j
                  ps = ps_h.tile([P, NTOK], f32, tag="psH")
                  for k in range(KD):
                      nc.tensor.matmul(
                          ps[:], wt[k][:, j * P:(j + 1) * P],
                          xT[:, k, :],
                          start=(k == 0), stop=(k == KD - 1),
                      )
                  if is_first:
                      act_silu(g1s[:, jj, :], ps[:])
                  else:
                      nc.vector.tensor_tensor(
                          g1s[:, jj, :], ps[:], g1s[:, jj, :], AluOpType.mult
                      )
      # second matmul of shared expert initializes the accumulator
      for ch in range(2):
          wt = []
          for k in range(KS):
              w = wk.tile([P, D // 2], MMDT, tag="wk")
              dma.dma_start(
                  out=w[:],
                  in_=dram["w2"][k * P:(k + 1) * P,
                                 ch * (D // 2):(ch + 1) * (D // 2)],
              )
              wt.append(w)
          for t in range(NT):
              ps = ps_o.tile([P, D // 2], f32, tag="psO")
              for k in range(KS):
                  nc.tensor.matmul(
                      ps[:], g1s[:, k, t * P:(t + 1) * P], wt[k][:],
                      start=(k == 0), stop=(k == KS - 1),
                  )
              nc.vector.tensor_scalar(
                  out=routed[:, t, ch * (D // 2):(ch + 1) * (D // 2)],
                  in0=ps[:], scalar1=OUT_SCALE, scalar2=None, op0=AluOpType.mult,
              )

    # ---- routed experts (dense, gated accumulate) ----
    for e in (range(E) if "exp" in PARTS else []):
        wt1 = []
        for k in range(KD):
            w = wk.tile([P, HE], MMDT, tag="wk")
            dma.dma_start(out=w[:], in_=dram["W1e"][e, k * P:(k + 1) * P, :])
            wt1.append(w)
        hs = hp.tile([P, KH, NTOK], MMDT, tag="hs")
        for j in range(KH):
            ps = ps_h.tile([P, NTOK], f32, tag="psH")
            for k in range(KD):
                nc.tensor.matmul(
                    ps[:], wt1[k][:, j * P:(j + 1) * P],
                    xT[:, k, :],
                    start=(k == 0), stop=(k == KD - 1),
                )
            act_gelu(hs[:, j, :], ps[:])
        wt2 = []
        for k in range(KH):
            w = wk.tile([P, D], MMDT, tag="wk")
            dma.dma_start(out=w[:], in_=dram["W2e"][e, k * P:(k + 1) * P, :])
            wt2.append(w)
        for t in range(NT):
            for ch in range(2):
                ps = ps_o.tile([P, D // 2], f32, tag="psO")
                for k in range(KH):
                    nc.tensor.matmul(
                        ps[:], hs[:, k, t * P:(t + 1) * P],
                        wt2[k][:, ch * (D // 2):(ch + 1) * (D // 2)],
                        start=(k == 0), stop=(k == KH - 1),
                    )
                csl = slice(ch * (D // 2), (ch + 1) * (D // 2))
                nc.vector.scalar_tensor_tensor(
                    out=routed[:, t, csl], in0=ps[:],
                    scalar=comb[:, t * E + e:t * E + e + 1],
                    in1=routed[:, t, csl],
                    op0=AluOpType.mult, op1=AluOpType.add,
                )

    dma.dma_start(
        out=dram["out"].rearrange("(g p) c -> p g c", p=P), in_=routed[:]
    )


def build_nc(reps=1):
    nc = bacc.Bacc(None, target_bir_lowering=False, debug=False)
    dram = {
        "xT": nc.dram_tensor("xT", [D, NTOK], MMDT, kind="ExternalInput").ap(),
        "x_rt": nc.dram_tensor("x_rt", [D, NTOK], f32, kind="ExternalInput").ap(),
        "t_row": nc.dram_tensor("t_row", [D], f32, kind="ExternalInput").ap(),
        "router_W": nc.dram_tensor(
            "router_W", [2 * D, E], f32, kind="ExternalInput").ap(),
        "router_bias_b": nc.dram_tensor(
            "router_bias_b", [P, NT * E], f32, kind="ExternalInput").ap(),
        "w1": nc.dram_tensor("w1", [D, HS], MMDT, kind="ExternalInput").ap(),
        "w3": nc.dram_tensor("w3", [D, HS], MMDT, kind="ExternalInput").ap(),
        "w2": nc.dram_tensor("w2", [HS, D], MMDT, kind="ExternalInput").ap(),
        "W1e": nc.dram_tensor("W1e", [E, D, HE], MMDT, kind="ExternalInput").ap(),
        "W2e": nc.dram_tensor("W2e", [E, HE, D], MMDT, kind="ExternalInput").ap(),
        "out": nc.dram_tensor("out", [NTOK, D], f32, kind="ExternalOutput").ap(),
    }
    with tile.TileContext(nc) as tc:
        with ExitStack() as ctx:
            const = ctx.enter_context(tc.tile_pool(name="const", bufs=1))
            xp = ctx.enter_context(tc.tile_pool(name="xp", bufs=1))
            tp = ctx.enter_context(tc.tile_pool(name="tp", bufs=1))
            wk = ctx.enter_context(tc.tile_pool(name="wk", bufs=18))
            g1p = ctx.enter_context(tc.tile_pool(name="g1p", bufs=1))
            hp = ctx.enter_context(tc.tile_pool(name="hp", bufs=2))
            accp = ctx.enter_context(tc.tile_pool(name="accp", bufs=1))
            rp = ctx.enter_context(tc.tile_pool(name="rp", bufs=1))
            tmpp = (ctx.enter_context(tc.tile_pool(name="tmpp", bufs=2))
                    if ACT_COMPAT else None)
            xrp = ctx.enter_context(tc.tile_pool(name="xrp", bufs=2))
            ps_h = ctx.enter_context(tc.tile_pool(name="ps_h", bufs=2, space="PSUM"))
            ps_o = ctx.enter_context(tc.tile_pool(name="ps_o", bufs=3, space="PSUM"))
            ps_r = ctx.enter_context(tc.tile_pool(name="ps_r", bufs=1, space="PSUM"))
            pools = (xp, tp, wk, g1p, hp, accp, rp, tmpp, xrp, ps_h, ps_o, ps_r)

            identity_tile = const.tile([P, P], f32, tag="ident")
            make_identity(nc, identity_tile[:])

            if reps == 1:
                emit_body(nc, pools, dram, identity_tile)
            else:
                with tc.For_i(0, reps, 1):
                    emit_body(nc, pools, dram, identity_tile)
    nc.compile()
    return nc


def _np_mmdt():
    if MM_MODE == "bf16":
        import ml_dtypes
        return np.dtype(ml_dtypes.bfloat16)
    return np.dtype(np.float32)


def make_in_maps(x, t_emb, router_W, router_bias, w1, w3, w2, W1e, W2e):
    mmnp = _np_mmdt()
    xf = np.ascontiguousarray(x, dtype=np.float32).reshape(B * T, D)
    bias_b = np.ascontiguousarray(
        np.tile(np.asarray(router_bias, np.float32)[None, :], (P, NT))
    )
    shared = {
        "router_W": np.ascontiguousarray(router_W, np.float32),
        "router_bias_b": bias_b,
        "w1": np.ascontiguousarray(w1, mmnp),
        "w3": np.ascontiguousarray(w3, mmnp),
        "w2": np.ascontiguousarray(w2, mmnp),
        "W1e": np.ascontiguousarray(W1e, mmnp),
        "W2e": np.ascontiguousarray(W2e, mmnp),
    }
    in_maps = []
    for c in range(NCORES):
        chunk = xf[c * NTOK:(c + 1) * NTOK]  # [512, 1024]
        b = (c * NTOK) // T
        xT = np.ascontiguousarray(chunk.T)  # [1024, 512]
        t_row = np.ascontiguousarray(np.asarray(t_emb, np.float32)[b])
        in_maps.append({"xT": xT.astype(mmnp), "x_rt": xT, "t_row": t_row,
                        **shared})
    return in_maps


_NC_CACHE = {}


def get_nc(reps=1):
    if reps not in _NC_CACHE:
        _NC_CACHE[reps] = build_nc(reps)
    return _NC_CACHE[reps]


def kernel(x, t_emb, router_W, router_bias, w1, w3, w2, W1e, W2e):
    nc = get_nc(reps=1)
    in_maps = make_in_maps(x, t_emb, router_W, router_bias, w1, w3, w2, W1e, W2e)
    r = run_bass_kernel_spmd(nc, in_maps, list(range(NCORES)), trace=False)
    out = np.concatenate([r.results[c]["out"] for c in range(NCORES)], axis=0)
    return out.reshape(B, T, D).astype(np.float32)

